# revision 5
# baseline (speedup 1.0000x reference)
"""Dual cross-attention (nn_Cross_Attention_Layer) Trainium2 Bass kernel.

Reference computation (N=4096, D=2048, fp32):
    Q_t/K_t/V_t = inputs_t @ W{q,k,v}_t.T ; same for _d
    alpha_t = softmax(mask ? Q_d @ K_t.T : NEG) ; out_t = alpha_t @ V_t
    alpha_d = softmax(mask ? Q_t @ K_d.T : NEG) ; out_d = alpha_d @ V_d
    mask[i, j] = j < lens[i]

Sharding: rows (queries) split across 8 cores, 512 rows each.  The score
and output matmuls are reassociated so no core ever materializes full
K/V projections:
    scores_t = (Q_d_slab @ Wk_t) @ inputs_t.T
    out_t    = (alpha_t @ inputs_t) @ Wv_t.T
which partitions the total FLOPs exactly 8 ways with no collectives.

All matmuls run as float32r (tf32-like) at full PE rate; softmax is
fp32 (exact max subtraction, exp on ScalarE with accumulated row-sum;
1/sum folded into the final output eviction as a per-partition scale).
"""

import sys

for _p in ("/opt/pypackages", "/opt/trn_rl_repo"):
    if _p not in sys.path:
        sys.path.insert(0, _p)

from contextlib import ExitStack

import numpy as np

import concourse.bass as bass
import concourse.mybir as mybir
import concourse.tile as tile
from concourse import bacc
from concourse.bass_utils import run_bass_kernel_spmd
from concourse.masks import make_identity

F32 = mybir.dt.float32
F32R = mybir.dt.float32r
U8 = mybir.dt.uint8

N = 4096          # sequence length
D = 2048          # hidden dim
NCORES = 8
R = N // NCORES   # rows (queries) per core = 512
P = 128           # partitions
KT = D // P       # contraction tiles over D = 16
MT = R // P       # row tiles per slab = 4
JC = N // 512     # 512-wide column chunks of the score matrix = 8
JT = N // P       # 128-wide column tiles of the score matrix = 32
NEG = -2.0 ** 31


def _emit_side(nc, tc, stack, side, wq_t_ap, wk_ap, xslabT_ap, xT_ap, x_ap,
               wvT_ap, out_ap, mask_tiles, neg_tile, ident):
    """Emit one attention side (t or d). APs are DRAM access patterns."""

    # ---- Stage A: Q.T [od, i] = (xslab @ Wq.T).T --------------------
    es_q = ExitStack()
    p_q = es_q.enter_context(tc.tile_pool(name=f"q_{side}", bufs=1, side="right"))
    q_tiles = []
    with tc.tile_pool(name=f"a_in_{side}", bufs=2, side="right") as p_ain, \
         tc.tile_pool(name=f"a_x_{side}", bufs=1, side="right") as p_ax, \
         tc.tile_pool(name=f"a_ps_{side}", bufs=8, space="PSUM") as p_aps:
        xs_tiles = []
        for k in range(KT):
            xs = p_ax.tile([P, R], F32R, name=f"xs_{side}_{k}", tag="xs", bufs=KT)
            nc.gpsimd.dma_start(xs[:], xslabT_ap[k * P:(k + 1) * P, :])
            xs_tiles.append(xs)
        for h in range(2):
            psl = []
            for mm in range(8):
                pq = p_aps.tile([P, R], F32, name=f"apq_{side}_{h}_{mm}",
                                tag="apq")
                psl.append(pq)
            for k in range(KT):
                wq = p_ain.tile([P, 1024], F32R, name=f"wqh_{side}_{h}_{k}",
                                tag="wqh")
                nc.gpsimd.dma_start(
                    wq[:], wq_t_ap[k * P:(k + 1) * P, h * 1024:(h + 1) * 1024])
                for mm in range(8):
                    nc.tensor.matmul(
                        psl[mm][:], wq[:, mm * P:(mm + 1) * P], xs_tiles[k][:],
                        start=(k == 0), stop=(k == KT - 1))
            for mm in range(8):
                q = p_q.tile([P, R], F32R, name=f"q_{side}_{h}_{mm}", tag="q",
                             bufs=16)
                nc.scalar.copy(q[:], psl[mm][:])
                q_tiles.append(q)

    # ---- Stage B: QM.T [d2, i] = (Q @ Wk).T -------------------------
    es_qm = ExitStack()
    p_qm = es_qm.enter_context(tc.tile_pool(name=f"qm_{side}", bufs=1, side="left"))
    qm_tiles = []
    with tc.tile_pool(name=f"b_in_{side}", bufs=2, side="left") as p_bin, \
         tc.tile_pool(name=f"b_ps_{side}", bufs=8, space="PSUM") as p_bps:
        for h in range(2):
            psl = []
            for mm in range(8):
                pb = p_bps.tile([P, R], F32, name=f"bps_{side}_{h}_{mm}",
                                tag="bps")
                psl.append(pb)
            for k in range(KT):
                wk = p_bin.tile([P, 1024], F32R, name=f"wkh_{side}_{h}_{k}",
                                tag="wkh")
                nc.gpsimd.dma_start(
                    wk[:], wk_ap[k * P:(k + 1) * P, h * 1024:(h + 1) * 1024])
                for mm in range(8):
                    nc.tensor.matmul(
                        psl[mm][:], wk[:, mm * P:(mm + 1) * P], q_tiles[k][:],
                        start=(k == 0), stop=(k == KT - 1))
            for mm in range(8):
                qm = p_qm.tile([P, R], F32R, name=f"qm_{side}_{h}_{mm}",
                               tag="qm", bufs=16)
                nc.scalar.copy(qm[:], psl[mm][:])
                qm_tiles.append(qm)

    es_q.close()  # Q tiles are dead once B is emitted

    # ---- Stage C: scores [i, j] = QM @ x.T + mask + chunk max -------
    es_sc = ExitStack()
    p_stat = stack.enter_context(tc.tile_pool(name=f"stat_{side}", bufs=1, side="right"))
    p_sc = es_sc.enter_context(tc.tile_pool(name=f"sc_{side}", bufs=1, side="right"))
    sc = [p_sc.tile([P, N], F32, name=f"sc_{side}_{m}", tag=f"sc{m}")
          for m in range(MT)]
    cmax = [p_stat.tile([P, JC], F32, name=f"cmax_{side}_{m}", tag=f"cm{m}")
            for m in range(MT)]
    csum = [p_stat.tile([P, JC], F32, name=f"csum_{side}_{m}", tag=f"cs{m}")
            for m in range(MT)]
    negmax = [p_stat.tile([P, 1], F32, name=f"negmax_{side}_{m}", tag=f"nm{m}")
              for m in range(MT)]
    sumv = [p_stat.tile([P, 1], F32, name=f"sumv_{side}_{m}", tag=f"sv{m}")
            for m in range(MT)]
    recip = [p_stat.tile([P, 1], F32, name=f"recip_{side}_{m}", tag=f"rc{m}")
             for m in range(MT)]
    with tc.tile_pool(name=f"c_in_{side}", bufs=3, side="right") as p_cin, \
         tc.tile_pool(name=f"c_ps_{side}", bufs=8, space="PSUM") as p_cps:
        for jc in range(JC):
            psl = []
            for m in range(MT):
                pc = p_cps.tile([P, 512], F32, name=f"cps_{side}_{jc}_{m}",
                                tag="cps")
                psl.append(pc)
            for k in range(KT):
                xt = p_cin.tile([P, 512], F32R, name=f"cxt_{side}_{jc}_{k}",
                                tag="cxt")
                nc.gpsimd.dma_start(
                    xt[:], xT_ap[k * P:(k + 1) * P, jc * 512:(jc + 1) * 512])
                for m in range(MT):
                    nc.tensor.matmul(
                        psl[m][:], qm_tiles[k][:, m * P:(m + 1) * P], xt[:],
                        start=(k == 0), stop=(k == KT - 1))
            for m in range(MT):
                s_ap = sc[m][:, jc * 512:(jc + 1) * 512]
                nc.scalar.copy(s_ap, psl[m][:])
                nc.vector.copy_predicated(
                    s_ap, mask_tiles[m][:, jc * 512:(jc + 1) * 512], neg_tile[:])
                nc.vector.tensor_reduce(
                    out=cmax[m][:, jc:jc + 1], in_=s_ap,
                    op=mybir.AluOpType.max, axis=mybir.AxisListType.X)

    es_qm.close()  # QM tiles are dead once C is emitted

    # ---- softmax + transpose into alphaT [j, i] ---------------------
    es_at = ExitStack()
    p_at = es_at.enter_context(tc.tile_pool(name=f"at_{side}", bufs=1, side="left"))
    at_tiles = [p_at.tile([P, R], F32R, name=f"at_{side}_{j}", tag="at",
                          bufs=JT) for j in range(JT)]
    with tc.tile_pool(name=f"t_ps_{side}", bufs=8, space="PSUM") as p_tps:
        for m in range(MT):
            nc.vector.tensor_reduce(
                out=negmax[m][:], in_=cmax[m][:], op=mybir.AluOpType.max,
                axis=mybir.AxisListType.X, negate=True)
        for m in range(MT):
            for jc in range(JC):
                s_ap = sc[m][:, jc * 512:(jc + 1) * 512]
                nc.scalar.activation(
                    s_ap, s_ap, mybir.ActivationFunctionType.Exp,
                    bias=negmax[m][:], scale=1.0,
                    accum_out=csum[m][:, jc:jc + 1])
                for s in range(4):
                    jt = jc * 4 + s
                    pt = p_tps.tile([P, P], F32, name=f"tps_{side}_{m}_{jt}",
                                    tag="tps")
                    nc.tensor.transpose(
                        pt[:], sc[m][:, jt * P:(jt + 1) * P], ident[:])
                    nc.scalar.copy(at_tiles[jt][:, m * P:(m + 1) * P], pt[:])
        for m in range(MT):
            nc.vector.tensor_reduce(
                out=sumv[m][:], in_=csum[m][:], op=mybir.AluOpType.add,
                axis=mybir.AxisListType.X)
            nc.vector.reciprocal(recip[m][:], sumv[m][:])

    es_sc.close()  # score slab dead once transposes are emitted

    # ---- Stage D: U.T [d, i] = x.T-contract with alphaT -------------
    p_u = stack.enter_context(tc.tile_pool(name=f"u_{side}", bufs=1, side="right"))
    u_tiles = []
    with tc.tile_pool(name=f"d_in_{side}", bufs=3, side="left") as p_din, \
         tc.tile_pool(name=f"d_ps_{side}", bufs=8, space="PSUM") as p_dps:
        for h in range(2):
            psl = []
            for dt in range(8):
                pd = p_dps.tile([P, R], F32, name=f"dps_{side}_{h}_{dt}",
                                tag="dps")
                psl.append(pd)
            for j in range(JT):
                xr = p_din.tile([P, 1024], F32R, name=f"dxr_{side}_{h}_{j}",
                                tag="dxr")
                nc.gpsimd.dma_start(
                    xr[:], x_ap[j * P:(j + 1) * P, h * 1024:(h + 1) * 1024])
                for dt in range(8):
                    nc.tensor.matmul(
                        psl[dt][:], xr[:, dt * P:(dt + 1) * P], at_tiles[j][:],
                        start=(j == 0), stop=(j == JT - 1))
            for dt in range(8):
                u = p_u.tile([P, R], F32R, name=f"u_{side}_{h}_{dt}", tag="u",
                             bufs=16)
                nc.scalar.copy(u[:], psl[dt][:])
                u_tiles.append(u)

    es_at.close()  # alphaT dead once D is emitted

    # ---- Stage E: out [i, o] = (U @ Wv.T) * recip -------------------
    with tc.tile_pool(name=f"e_in_{side}", bufs=3, side="left") as p_ein, \
         tc.tile_pool(name=f"e_out_{side}", bufs=8, side="left") as p_eout, \
         tc.tile_pool(name=f"e_ps_{side}", bufs=8, space="PSUM") as p_eps:
        for oc in range(4):
            psl = []
            for m in range(MT):
                pe = p_eps.tile([P, 512], F32, name=f"eps_{side}_{oc}_{m}",
                                tag="eps")
                psl.append(pe)
            for k in range(KT):
                wv = p_ein.tile([P, 512], F32R, name=f"ewv_{side}_{oc}_{k}",
                                tag="ewv")
                nc.gpsimd.dma_start(
                    wv[:], wvT_ap[k * P:(k + 1) * P, oc * 512:(oc + 1) * 512])
                for m in range(MT):
                    nc.tensor.matmul(
                        psl[m][:], u_tiles[k][:, m * P:(m + 1) * P], wv[:],
                        start=(k == 0), stop=(k == KT - 1))
            for m in range(MT):
                ot = p_eout.tile([P, 512], F32, name=f"eo_{side}_{oc}_{m}",
                                 tag="eo")
                nc.scalar.mul(ot[:], psl[m][:], recip[m][:])
                nc.sync.dma_start(
                    out_ap[m * P:(m + 1) * P, oc * 512:(oc + 1) * 512], ot[:])


def build_program():
    nc = bacc.Bacc("TRN2", target_bir_lowering=False, debug=False,
                   num_devices=NCORES)

    def din(name, shape, dt=F32):
        return nc.dram_tensor(name, shape, dt, kind="ExternalInput").ap()

    aps = {
        "xslabT_d": din("xslabT_d", [D, R]),
        "xslabT_t": din("xslabT_t", [D, R]),
        "wqdT": din("wqdT", [D, D]),
        "wqtT": din("wqtT", [D, D]),
        "wkt": din("wkt", [D, D]),
        "wkd": din("wkd", [D, D]),
        "xtT": din("xtT", [D, N]),
        "xdT": din("xdT", [D, N]),
        "xt": din("xt", [N, D]),
        "xd": din("xd", [N, D]),
        "wvtT": din("wvtT", [D, D]),
        "wvdT": din("wvdT", [D, D]),
        "mask": din("mask", [R, N], U8),
    }
    out_t = nc.dram_tensor("out_t", [R, D], F32, kind="ExternalOutput").ap()
    out_d = nc.dram_tensor("out_d", [R, D], F32, kind="ExternalOutput").ap()

    with tile.TileContext(nc) as tc, ExitStack() as stack:
        p_const = stack.enter_context(tc.tile_pool(name="const", bufs=1))
        ident = p_const.tile([P, P], F32, name="ident", tag="ident")
        make_identity(nc, ident[:])
        neg_tile = p_const.tile([P, 512], F32, name="neg", tag="neg")
        nc.vector.memset(neg_tile[:], NEG)
        mask_tiles = []
        for m in range(MT):
            mk = p_const.tile([P, N], U8, name=f"mask_{m}", tag=f"mask{m}")
            nc.sync.dma_start(mk[:], aps["mask"][m * P:(m + 1) * P, :])
            mask_tiles.append(mk)

        with ExitStack() as st_t:
            _emit_side(nc, tc, st_t, "t", aps["wqdT"], aps["wkt"],
                       aps["xslabT_d"], aps["xtT"], aps["xt"], aps["wvtT"],
                       out_t, mask_tiles, neg_tile, ident)
        with ExitStack() as st_d:
            _emit_side(nc, tc, st_d, "d", aps["wqtT"], aps["wkd"],
                       aps["xslabT_t"], aps["xdT"], aps["xd"], aps["wvdT"],
                       out_d, mask_tiles, neg_tile, ident)

    nc.compile()
    return nc


_NC_CACHE = None


def _get_program():
    global _NC_CACHE
    if _NC_CACHE is None:
        _NC_CACHE = build_program()
    return _NC_CACHE


def kernel(inputs_t, inputs_d, Wq_t, Wk_t, Wv_t, Wq_d, Wk_d, Wv_d, lens,
           _trace=False):
    inputs_t = np.ascontiguousarray(np.asarray(inputs_t, dtype=np.float32))
    inputs_d = np.ascontiguousarray(np.asarray(inputs_d, dtype=np.float32))
    lens_np = np.asarray(lens)
    out_dtype_lens = lens_np.dtype  # preserved implicitly; lens is input-only

    def t(a):
        return np.ascontiguousarray(np.asarray(a, dtype=np.float32).T)

    wqdT, wqtT = t(Wq_d), t(Wq_t)
    wvtT, wvdT = t(Wv_t), t(Wv_d)
    wkt = np.ascontiguousarray(np.asarray(Wk_t, dtype=np.float32))
    wkd = np.ascontiguousarray(np.asarray(Wk_d, dtype=np.float32))
    xtT, xdT = t(inputs_t), t(inputs_d)

    j_idx = np.arange(N)
    in_maps = []
    for c in range(NCORES):
        rows = slice(c * R, (c + 1) * R)
        mask = (j_idx[None, :] >= np.asarray(lens_np[rows]).reshape(-1, 1))
        in_maps.append({
            "xslabT_d": np.ascontiguousarray(inputs_d[rows].T),
            "xslabT_t": np.ascontiguousarray(inputs_t[rows].T),
            "wqdT": wqdT, "wqtT": wqtT,
            "wkt": wkt, "wkd": wkd,
            "xtT": xtT, "xdT": xdT,
            "xt": inputs_t, "xd": inputs_d,
            "wvtT": wvtT, "wvdT": wvdT,
            "mask": np.ascontiguousarray(mask.astype(np.uint8)),
        })

    nc = _get_program()
    res = run_bass_kernel_spmd(nc, in_maps, list(range(NCORES)), trace=_trace)
    out_t = np.concatenate([res.results[c]["out_t"] for c in range(NCORES)], axis=0)
    out_d = np.concatenate([res.results[c]["out_d"] for c in range(NCORES)], axis=0)
    if _trace:
        kernel.last_exec_time_ns = res.exec_time_ns
        kernel.last_results = res
    return (out_t, out_d)


# revision 7
# speedup vs baseline: 1.7004x; 1.7004x over previous
"""Dual cross-attention (nn_Cross_Attention_Layer) Trainium2 Bass kernel.

Reference computation (N=4096, D=2048, fp32):
    Q_t/K_t/V_t = inputs_t @ W{q,k,v}_t.T ; same for _d
    alpha_t = softmax(mask ? Q_d @ K_t.T : NEG) ; out_t = alpha_t @ V_t
    alpha_d = softmax(mask ? Q_t @ K_d.T : NEG) ; out_d = alpha_d @ V_d
    mask[i, j] = j < lens[i]

Sharding: rows (queries) split across 8 cores, 512 rows each.  The score
and output matmuls are reassociated so no core ever materializes full
K/V projections:
    scores_t = (Q_d_slab @ Wk_t) @ inputs_t.T
    out_t    = (alpha_t @ inputs_t) @ Wv_t.T
which partitions the total FLOPs exactly 8 ways with no collectives.

All matmuls run as float32r (tf32-like multiply, fp32 accumulate) at the
full PE rate; softmax is fp32 (exact max subtraction, exp on ScalarE
with accumulated row-sum; 1/sum folded into the final output eviction
as a per-partition scale).  Inputs are declared float32r in DRAM so
loads ride the HWDGE (sync-engine) path with no cast.  A single shared
PSUM pool (all tiles bank-shaped, one tag) lets consecutive stages
rotate through the 8 banks without pool-boundary stalls.
"""

import sys

for _p in ("/opt/pypackages", "/opt/trn_rl_repo"):
    if _p not in sys.path:
        sys.path.insert(0, _p)

from contextlib import ExitStack

import numpy as np

import concourse.bass as bass
import concourse.mybir as mybir
import concourse.tile as tile
from concourse import bacc
from concourse.bass_utils import run_bass_kernel_spmd
from concourse.masks import make_identity

F32 = mybir.dt.float32
F32R = mybir.dt.float32r
U8 = mybir.dt.uint8

N = 4096          # sequence length
D = 2048          # hidden dim
NCORES = 8
R = N // NCORES   # rows (queries) per core = 512
P = 128           # partitions
KT = D // P       # contraction tiles over D = 16
MT = R // P       # row tiles per slab = 4
JC = N // 512     # 512-wide column chunks of the score matrix = 8
JT = N // P       # 128-wide column tiles of the score matrix = 32
NEG = -2.0 ** 31


def _emit_side(nc, tc, stack, side, wq_t_ap, wk_ap, xslabT_ap, xT_ap, x_ap,
               wvT_ap, out_ap, mask_tiles, neg_tile, ident, p_ps):
    """Emit one attention side (t or d). APs are DRAM access patterns."""
    wq3 = wq_t_ap.rearrange("(kt p) m -> kt p m", p=P)
    wk3 = wk_ap.rearrange("(kt p) m -> kt p m", p=P)
    xT3 = xT_ap.rearrange("(kt p) m -> kt p m", p=P)
    x3 = x_ap.rearrange("(kt p) m -> kt p m", p=P)
    wv3 = wvT_ap.rearrange("(kt p) m -> kt p m", p=P)
    xs3 = xslabT_ap.rearrange("(kt p) m -> kt p m", p=P)

    def ps_tile(nm):
        return p_ps.tile([P, 512], F32, name=f"{nm}_{side}", tag="ps")

    # ---- Stage A: Q.T [od, i] = (xslab @ Wq.T).T --------------------
    es_q = ExitStack()
    p_q = es_q.enter_context(tc.tile_pool(name=f"q_{side}", bufs=1, side="right"))
    q_tiles = []
    with tc.tile_pool(name=f"a_in_{side}", bufs=6, side="right") as p_ain, \
         tc.tile_pool(name=f"a_x_{side}", bufs=1, side="right") as p_ax:
        xs_tiles = []
        for k in range(KT):
            xs = p_ax.tile([P, R], F32R, name=f"xs_{side}_{k}", tag="xs",
                           bufs=KT)
            nc.sync.dma_start(xs[:], xs3[k])
            xs_tiles.append(xs)
        for h in range(2):
            psl = [ps_tile(f"apq{h}{mm}") for mm in range(8)]
            for k in range(KT):
                wq = p_ain.tile([P, 1024], F32R, name=f"wqh_{side}_{h}_{k}",
                                tag="wqh")
                nc.sync.dma_start(wq[:], wq3[k, :, h * 1024:(h + 1) * 1024])
                for mm in range(8):
                    nc.tensor.matmul(
                        psl[mm][:], wq[:, mm * P:(mm + 1) * P], xs_tiles[k][:],
                        start=(k == 0), stop=(k == KT - 1))
            for mm in range(8):
                q = p_q.tile([P, R], F32R, name=f"q_{side}_{h}_{mm}", tag="q",
                             bufs=16)
                nc.scalar.copy(q[:], psl[mm][:])
                q_tiles.append(q)

    # ---- Stage B: QM.T [d2, i] = (Q @ Wk).T -------------------------
    es_qm = ExitStack()
    p_qm = es_qm.enter_context(tc.tile_pool(name=f"qm_{side}", bufs=1, side="left"))
    qm_tiles = []
    with tc.tile_pool(name=f"b_in_{side}", bufs=6, side="left") as p_bin:
        for h in range(2):
            psl = [ps_tile(f"bps{h}{mm}") for mm in range(8)]
            for k in range(KT):
                wk = p_bin.tile([P, 1024], F32R, name=f"wkh_{side}_{h}_{k}",
                                tag="wkh")
                nc.sync.dma_start(wk[:], wk3[k, :, h * 1024:(h + 1) * 1024])
                for mm in range(8):
                    nc.tensor.matmul(
                        psl[mm][:], wk[:, mm * P:(mm + 1) * P], q_tiles[k][:],
                        start=(k == 0), stop=(k == KT - 1))
            for mm in range(8):
                qm = p_qm.tile([P, R], F32R, name=f"qm_{side}_{h}_{mm}",
                               tag="qm", bufs=16)
                nc.scalar.copy(qm[:], psl[mm][:])
                qm_tiles.append(qm)
    es_q.close()  # Q tiles are dead once B is emitted

    # ---- Stage C: scores [i, j] = QM @ x.T + mask + chunk max -------
    es_sc = ExitStack()
    p_stat = stack.enter_context(
        tc.tile_pool(name=f"stat_{side}", bufs=1, side="right"))
    p_sc = es_sc.enter_context(
        tc.tile_pool(name=f"sc_{side}", bufs=1, side="right"))
    sc = [p_sc.tile([P, N], F32, name=f"sc_{side}_{m}", tag=f"sc{m}")
          for m in range(MT)]
    cmax = [p_stat.tile([P, JC], F32, name=f"cmax_{side}_{m}", tag=f"cm{m}")
            for m in range(MT)]
    csum = [p_stat.tile([P, JC], F32, name=f"csum_{side}_{m}", tag=f"cs{m}")
            for m in range(MT)]
    negmax = [p_stat.tile([P, 1], F32, name=f"negmax_{side}_{m}", tag=f"nm{m}")
              for m in range(MT)]
    sumv = [p_stat.tile([P, 1], F32, name=f"sumv_{side}_{m}", tag=f"sv{m}")
            for m in range(MT)]
    recip = [p_stat.tile([P, 1], F32, name=f"recip_{side}_{m}", tag=f"rc{m}")
             for m in range(MT)]
    with tc.tile_pool(name=f"c_in_{side}", bufs=8, side="right") as p_cin:
        for jc in range(JC):
            psl = [ps_tile(f"cps{jc}{m}") for m in range(MT)]
            for k in range(KT):
                xt = p_cin.tile([P, 512], F32R, name=f"cxt_{side}_{jc}_{k}",
                                tag="cxt")
                nc.sync.dma_start(xt[:], xT3[k, :, jc * 512:(jc + 1) * 512])
                for m in range(MT):
                    nc.tensor.matmul(
                        psl[m][:], qm_tiles[k][:, m * P:(m + 1) * P], xt[:],
                        start=(k == 0), stop=(k == KT - 1))
            for m in range(MT):
                s_ap = sc[m][:, jc * 512:(jc + 1) * 512]
                nc.scalar.copy(s_ap, psl[m][:])
                nc.vector.copy_predicated(
                    s_ap, mask_tiles[m][:, jc * 512:(jc + 1) * 512], neg_tile[:])
                nc.vector.tensor_reduce(
                    out=cmax[m][:, jc:jc + 1], in_=s_ap,
                    op=mybir.AluOpType.max, axis=mybir.AxisListType.X)
    es_qm.close()  # QM tiles are dead once C is emitted

    # ---- softmax + transpose into alphaT [j, i] ---------------------
    es_at = ExitStack()
    p_at = es_at.enter_context(
        tc.tile_pool(name=f"at_{side}", bufs=1, side="left"))
    at_tiles = [p_at.tile([P, R], F32R, name=f"at_{side}_{j}", tag="at",
                          bufs=JT) for j in range(JT)]
    for m in range(MT):
        nc.vector.tensor_reduce(
            out=negmax[m][:], in_=cmax[m][:], op=mybir.AluOpType.max,
            axis=mybir.AxisListType.X, negate=True)
    for jc in range(JC):
        for m in range(MT):
            s_ap = sc[m][:, jc * 512:(jc + 1) * 512]
            nc.scalar.activation(
                s_ap, s_ap, mybir.ActivationFunctionType.Exp,
                bias=negmax[m][:], scale=1.0,
                accum_out=csum[m][:, jc:jc + 1])
            for s in range(4):
                jt = jc * 4 + s
                pt = ps_tile(f"tps{m}{jt}")
                nc.tensor.transpose(
                    pt[:, 0:P], sc[m][:, jt * P:(jt + 1) * P], ident[:])
                nc.scalar.copy(at_tiles[jt][:, m * P:(m + 1) * P], pt[:, 0:P])
    for m in range(MT):
        nc.vector.tensor_reduce(
            out=sumv[m][:], in_=csum[m][:], op=mybir.AluOpType.add,
            axis=mybir.AxisListType.X)
        nc.vector.reciprocal(recip[m][:], sumv[m][:])
    es_sc.close()  # score slab dead once transposes are emitted

    # ---- Stage D: U.T [d, i] = x.T-contract with alphaT -------------
    p_u = stack.enter_context(tc.tile_pool(name=f"u_{side}", bufs=1, side="right"))
    u_tiles = []
    with tc.tile_pool(name=f"d_in_{side}", bufs=6, side="left") as p_din:
        for h in range(2):
            psl = [ps_tile(f"dps{h}{dt}") for dt in range(8)]
            for j in range(JT):
                xr = p_din.tile([P, 1024], F32R, name=f"dxr_{side}_{h}_{j}",
                                tag="dxr")
                nc.sync.dma_start(xr[:], x3[j, :, h * 1024:(h + 1) * 1024])
                for dt in range(8):
                    nc.tensor.matmul(
                        psl[dt][:], xr[:, dt * P:(dt + 1) * P], at_tiles[j][:],
                        start=(j == 0), stop=(j == JT - 1))
            for dt in range(8):
                u = p_u.tile([P, R], F32R, name=f"u_{side}_{h}_{dt}", tag="u",
                             bufs=16)
                nc.scalar.copy(u[:], psl[dt][:])
                u_tiles.append(u)
    es_at.close()  # alphaT dead once D is emitted

    # ---- Stage E: out [i, o] = (U @ Wv.T) * recip -------------------
    with tc.tile_pool(name=f"e_in_{side}", bufs=8, side="left") as p_ein, \
         tc.tile_pool(name=f"e_out_{side}", bufs=1, side="left") as p_eout:
        eo = [p_eout.tile([P, D], F32, name=f"eo_{side}_{m}", tag=f"eo{m}")
              for m in range(MT)]
        for oc in range(4):
            psl = [ps_tile(f"eps{oc}{m}") for m in range(MT)]
            for k in range(KT):
                wv = p_ein.tile([P, 512], F32R, name=f"ewv_{side}_{oc}_{k}",
                                tag="ewv")
                nc.sync.dma_start(wv[:], wv3[k, :, oc * 512:(oc + 1) * 512])
                for m in range(MT):
                    nc.tensor.matmul(
                        psl[m][:], u_tiles[k][:, m * P:(m + 1) * P], wv[:],
                        start=(k == 0), stop=(k == KT - 1))
            for m in range(MT):
                nc.scalar.mul(eo[m][:, oc * 512:(oc + 1) * 512], psl[m][:],
                              recip[m][:])
        for m in range(MT):
            nc.gpsimd.dma_start(out_ap[m * P:(m + 1) * P, :], eo[m][:])


def build_program():
    nc = bacc.Bacc("TRN2", target_bir_lowering=False, debug=False,
                   num_devices=NCORES)

    def din(name, shape, dt=F32R):
        return nc.dram_tensor(name, shape, dt, kind="ExternalInput").ap()

    aps = {
        "xslabT_d": din("xslabT_d", [D, R]),
        "xslabT_t": din("xslabT_t", [D, R]),
        "wqdT": din("wqdT", [D, D]),
        "wqtT": din("wqtT", [D, D]),
        "wkt": din("wkt", [D, D]),
        "wkd": din("wkd", [D, D]),
        "xtT": din("xtT", [D, N]),
        "xdT": din("xdT", [D, N]),
        "xt": din("xt", [N, D]),
        "xd": din("xd", [N, D]),
        "wvtT": din("wvtT", [D, D]),
        "wvdT": din("wvdT", [D, D]),
        "mask": din("mask", [R, N], U8),
    }
    out_t = nc.dram_tensor("out_t", [R, D], F32, kind="ExternalOutput").ap()
    out_d = nc.dram_tensor("out_d", [R, D], F32, kind="ExternalOutput").ap()

    with tile.TileContext(nc) as tc, ExitStack() as stack:
        p_const = stack.enter_context(tc.tile_pool(name="const", bufs=1))
        p_ps = stack.enter_context(
            tc.tile_pool(name="ps", bufs=8, space="PSUM"))
        ident = p_const.tile([P, P], F32, name="ident", tag="ident")
        make_identity(nc, ident[:])
        neg_tile = p_const.tile([P, 512], F32, name="neg", tag="neg")
        nc.vector.memset(neg_tile[:], NEG)
        mask_tiles = []
        for m in range(MT):
            mk = p_const.tile([P, N], U8, name=f"mask_{m}", tag=f"mask{m}")
            nc.gpsimd.dma_start(mk[:], aps["mask"][m * P:(m + 1) * P, :])
            mask_tiles.append(mk)

        with ExitStack() as st_t:
            _emit_side(nc, tc, st_t, "t", aps["wqdT"], aps["wkt"],
                       aps["xslabT_d"], aps["xtT"], aps["xt"], aps["wvtT"],
                       out_t, mask_tiles, neg_tile, ident, p_ps)
        with ExitStack() as st_d:
            _emit_side(nc, tc, st_d, "d", aps["wqtT"], aps["wkd"],
                       aps["xslabT_t"], aps["xdT"], aps["xd"], aps["wvdT"],
                       out_d, mask_tiles, neg_tile, ident, p_ps)

    nc.compile()
    return nc


_NC_CACHE = None


def _get_program():
    global _NC_CACHE
    if _NC_CACHE is None:
        _NC_CACHE = build_program()
    return _NC_CACHE


def kernel(inputs_t, inputs_d, Wq_t, Wk_t, Wv_t, Wq_d, Wk_d, Wv_d, lens,
           _trace=False):
    inputs_t = np.ascontiguousarray(np.asarray(inputs_t, dtype=np.float32))
    inputs_d = np.ascontiguousarray(np.asarray(inputs_d, dtype=np.float32))
    lens_np = np.asarray(lens)

    def t(a):
        return np.ascontiguousarray(np.asarray(a, dtype=np.float32).T)

    wqdT, wqtT = t(Wq_d), t(Wq_t)
    wvtT, wvdT = t(Wv_t), t(Wv_d)
    wkt = np.ascontiguousarray(np.asarray(Wk_t, dtype=np.float32))
    wkd = np.ascontiguousarray(np.asarray(Wk_d, dtype=np.float32))
    xtT, xdT = t(inputs_t), t(inputs_d)

    j_idx = np.arange(N)
    in_maps = []
    for c in range(NCORES):
        rows = slice(c * R, (c + 1) * R)
        mask = (j_idx[None, :] >= np.asarray(lens_np[rows]).reshape(-1, 1))
        in_maps.append({
            "xslabT_d": np.ascontiguousarray(inputs_d[rows].T),
            "xslabT_t": np.ascontiguousarray(inputs_t[rows].T),
            "wqdT": wqdT, "wqtT": wqtT,
            "wkt": wkt, "wkd": wkd,
            "xtT": xtT, "xdT": xdT,
            "xt": inputs_t, "xd": inputs_d,
            "wvtT": wvtT, "wvdT": wvdT,
            "mask": np.ascontiguousarray(mask.astype(np.uint8)),
        })

    nc = _get_program()
    res = run_bass_kernel_spmd(nc, in_maps, list(range(NCORES)), trace=_trace)
    out_t = np.concatenate([res.results[c]["out_t"] for c in range(NCORES)], axis=0)
    out_d = np.concatenate([res.results[c]["out_d"] for c in range(NCORES)], axis=0)
    if _trace:
        kernel.last_exec_time_ns = res.exec_time_ns
        kernel.last_results = res
    return (out_t, out_d)
